# revision 1
# baseline (speedup 1.0000x reference)
"""Causal attention (single head, S=4096, d=1024) on 8 TRN2 NeuronCores —
collective-free formulation.

Core i computes output rows {i + 8m} (strided sequence-parallel Q). The
K/V AllGathers of the naive sharding are eliminated algebraically: with
K^T = Wk X^T and V = X Wv^T, and the full input X replicated to every
core as a kernel input (full_io),

    S = Q K^T = (Q Wk) X^T      (G := Q Wk is [512, 1024], local)
    O = A V   = (A X) Wv^T      (apply Wv once per core at the end)

so no inter-core communication is needed at all, and per-core matmul
work is unchanged: three [512x1024x1024] projections (Q, G, final Wv)
plus the causal scores/AV contractions. The output is produced
transposed (O^T) to keep the final projection's moving dim 512 wide;
the host assembles with a transpose.

Numerics: bf16 matmuls with f32 PSUM accumulation; softmax statistics
f32; exp skips max-subtraction (|q.k|/32 is bounded well inside bf16
range; exp of the additive -1e9 mask underflows to exactly 0). The
extra bf16 rounding of G adds ~sqrt(2)x score noise vs the direct
Q.K^T — well inside the error budget.
"""

import numpy as np
import ml_dtypes

import concourse.bass as bass  # noqa: F401  (registers engines)
import concourse.mybir as mybir
from concourse import bacc, tile, masks
from concourse.bass_utils import run_bass_kernel_spmd

SEQ = 4096
D = 1024
N_CORES = 8
CORE_IDS = list(range(N_CORES))
QLOC = SEQ // N_CORES          # 512 q rows per core
NQCH = QLOC // 128
OUT_SHAPE = (1024, 512)  # out dram tensor is O^T [D, QLOC]             # 4 q chunks of 128 rows
BF16 = mybir.dt.bfloat16
F32 = mybir.dt.float32
MASK_VAL = -1.0e9
SM_SCALE = 1.0 / np.sqrt(np.float32(D))
ACC_BUFS = 4
T_BUFS = 2
O_BUFS = 2


def _emit_compute(nc, tc, pp, cp_tiles, io, rep, variant="full"):
    ident, mask_sb = cp_tiles
    xq, xt, xn, wqT, wkN, wvT, out = io

    with tc.tile_pool(name="persist", bufs=1) as pers:
        g_sb = pers.tile([128, 8, QLOC], BF16, name="g_sb")    # G^T [din, q]
        axT_sb = pers.tile([128, 8, QLOC], BF16, name="axT_sb")  # (AX)^T
        ot_sb = pers.tile([128, 8, QLOC], F32, name="ot_sb")    # O^T
        sums_all = pers.tile([128, 4, 8], F32, name="sums_all")

        with tc.tile_pool(name="xt", bufs=1) as xtp:
            xt_sb = xtp.tile([128, 8, SEQ], BF16, name="xt_sb")  # X^T d-major
            xt_v = xt.rearrange("(a p) s -> p a s", p=128)
            for a in range(8):
                eng = nc.sync if a % 2 == 0 else nc.scalar
                eng.dma_start(xt_sb[:, a, :], xt_v[:, a, :])

            with tc.tile_pool(name="proj", bufs=1) as wp:
                xq_sb = wp.tile([128, 8, QLOC], BF16, name="xq_sb")
                q_sb = wp.tile([128, 8, QLOC], BF16, name="q_sb")
                wq_sb = wp.tile([128, 8, D], BF16, name="wq_sb")
                wkn_sb = wp.tile([128, 8, D], BF16, name="wkn_sb")
                nc.sync.dma_start(xq_sb[:],
                                  xq.rearrange("(a p) s -> p a s", p=128))
                nc.sync.dma_start(wq_sb[:],
                                  wqT.rearrange("(a p) n -> p a n", p=128))
                nc.scalar.dma_start(wkn_sb[:],
                                    wkN.rearrange("(a p) n -> p a n", p=128))

                # --- Q^T (strided rows) = Wq @ x_q^T : [1024 do, 512 q]
                for do in range(8):
                    ps = pp.tile([128, QLOC], F32, tag="acc", bufs=ACC_BUFS,
                                 name=f"ps_q{do}")
                    for di in range(8):
                        nc.tensor.matmul(
                            ps[:], wq_sb[:, di, 128 * do:128 * (do + 1)],
                            xq_sb[:, di, :], start=(di == 0), stop=(di == 7),
                        )
                    nc.vector.tensor_copy(q_sb[:, do, :], ps[:])

                # --- G^T = Wk^T @ Q^T : [1024 din, 512 q]
                # lhsT = Wk[do, di] slices (row-major Wk input), rhs = Q^T.
                for gi in range(8):
                    ps = pp.tile([128, QLOC], F32, tag="acc", bufs=ACC_BUFS,
                                 name=f"ps_g{gi}")
                    for do in range(8):
                        nc.tensor.matmul(
                            ps[:], wkn_sb[:, do, 128 * gi:128 * (gi + 1)],
                            q_sb[:, do, :], start=(do == 0), stop=(do == 7),
                        )
                    nc.vector.tensor_copy(g_sb[:, gi, :], ps[:])

            if variant == "proj":
                o_dbg = pers.tile([128, 64], F32, tag="dbg", name=f"dbg{rep}")
                nc.vector.tensor_copy(o_dbg[:, 0:8], g_sb[:, 0, 0:8])
                nc.vector.tensor_copy(o_dbg[:, 8:16], xt_sb[:, 0, 0:8])
                nc.sync.dma_start(out[0:128, 0:64], o_dbg[:])
                return

            with (
                tc.tile_pool(name="xn", bufs=1) as xnp,
                tc.tile_pool(name="late", bufs=1) as lp,
                tc.tile_pool(name="attn", bufs=2) as ap,
            ):
                xn_sb = xnp.tile([128, 32, D], BF16, name="xn_sb")  # X seq-major
                xn_v = xn.rearrange("(blk p) d -> p blk d", p=128)
                for g in range(8):
                    eng = nc.sync if g % 2 == 0 else nc.scalar
                    eng.dma_start(xn_sb[:, 4 * g:4 * (g + 1), :],
                                  xn_v[:, 4 * g:4 * (g + 1), :])
                wv_sb = lp.tile([128, 8, D], BF16, name="wv_sb")
                nc.scalar.dma_start(wv_sb[:],
                                    wvT.rearrange("(a p) n -> p a n", p=128))

                for b in range(NQCH):
                    nkb = 2 * (b + 1)          # number of 512-wide k blocks
                    klen = 512 * nkb
                    a_sb = ap.tile([128, SEQ], BF16, tag="A", bufs=2,
                                   name=f"a_sb{b}")
                    at_sb = ap.tile([128, SEQ], BF16, tag="AT", bufs=1,
                                    name=f"at_sb{b}")
                    sums = sums_all[:, b, :]

                    # scores S = G X^T blockwise + exp
                    for kb in range(nkb):
                        ps_s = pp.tile([128, 512], F32, tag="acc",
                                       bufs=ACC_BUFS, name=f"ps_s{b}_{kb}")
                        for di in range(8):
                            nc.tensor.matmul(
                                ps_s[:], g_sb[:, di, 128 * b:128 * (b + 1)],
                                xt_sb[:, di, 512 * kb:512 * (kb + 1)],
                                start=(di == 0), stop=(di == 7),
                            )
                        if kb >= 2 * b:  # diagonal band: causal mask
                            j0 = 512 * (kb - 2 * b)
                            nc.vector.tensor_add(
                                ps_s[:], ps_s[:], mask_sb[:, j0:j0 + 512]
                            )
                        nc.scalar.activation(
                            a_sb[:, 512 * kb:512 * (kb + 1)], ps_s[:],
                            mybir.ActivationFunctionType.Exp,
                            scale=float(SM_SCALE),
                            accum_out=sums[:, kb:kb + 1],
                        )

                    # transpose A (PE) -> A^T for the AX matmul
                    for kb in range(nkb):
                        ps_t = pp.tile([128, 512], BF16, tag="t", bufs=T_BUFS,
                                       name=f"ps_t{b}_{kb}")
                        for cc in range(4):
                            nc.tensor.transpose(
                                ps_t[:, 128 * cc:128 * (cc + 1)],
                                a_sb[:, 512 * kb + 128 * cc:
                                     512 * kb + 128 * (cc + 1)],
                                ident[:],
                            )
                        nc.vector.tensor_copy(
                            at_sb[:, 512 * kb:512 * (kb + 1)], ps_t[:]
                        )

                    stot = ap.tile([128, 1], F32, tag="stot", name=f"stot{b}")
                    rinv = ap.tile([128, 1], F32, tag="rinv", name=f"rinv{b}")
                    nc.vector.reduce_sum(
                        out=stot[:], in_=sums[:, 0:nkb], axis=mybir.AxisListType.X
                    )
                    nc.vector.reciprocal(rinv[:], stot[:])

                    # AX = A @ X rows [0, klen), normalized by 1/rowsum
                    ax_sb = ap.tile([128, D], BF16, tag="ax", bufs=1,
                                    name=f"ax_sb{b}")
                    nkc = klen // 128
                    for h in range(2):
                        ps_o = pp.tile([128, 512], F32, tag="o", bufs=O_BUFS,
                                       name=f"ps_o{b}_{h}")
                        for kc in range(nkc):
                            nc.tensor.matmul(
                                ps_o[:], at_sb[:, 128 * kc:128 * (kc + 1)],
                                xn_sb[:, kc, 512 * h:512 * (h + 1)],
                                start=(kc == 0), stop=(kc == nkc - 1),
                            )
                        nc.vector.tensor_scalar_mul(
                            ax_sb[:, 512 * h:512 * (h + 1)], ps_o[:], rinv[:]
                        )

                    # transpose AX -> (AX)^T column block b
                    for g2 in range(2):
                        ps_t2 = pp.tile([128, 512], BF16, tag="t", bufs=T_BUFS,
                                        name=f"ps_t2{b}_{g2}")
                        for j in range(4):
                            nc.tensor.transpose(
                                ps_t2[:, 128 * j:128 * (j + 1)],
                                ax_sb[:, 512 * g2 + 128 * j:
                                      512 * g2 + 128 * (j + 1)],
                                ident[:],
                            )
                        for j in range(4):
                            nc.vector.tensor_copy(
                                axT_sb[:, 4 * g2 + j, 128 * b:128 * (b + 1)],
                                ps_t2[:, 128 * j:128 * (j + 1)],
                            )

                # --- O^T = Wv (AX)^T : [1024 do, 512 q]
                for do in range(8):
                    ps = pp.tile([128, QLOC], F32, tag="o", bufs=O_BUFS,
                                 name=f"ps_ot{do}")
                    for di in range(8):
                        nc.tensor.matmul(
                            ps[:], wv_sb[:, di, 128 * do:128 * (do + 1)],
                            axT_sb[:, di, :], start=(di == 0), stop=(di == 7),
                        )
                    nc.vector.tensor_copy(ot_sb[:, do, :], ps[:])
                out_v = out.rearrange("(a p) q -> p a q", p=128)
                nc.sync.dma_start(out_v[:, 0:4, :], ot_sb[:, 0:4, :])
                nc.scalar.dma_start(out_v[:, 4:8, :], ot_sb[:, 4:8, :])


def build_nc(reps=1, variant="full"):
    nc = bacc.Bacc("TRN2", target_bir_lowering=False)

    xq = nc.dram_tensor("xq", [D, QLOC], BF16, kind="ExternalInput")
    xt = nc.dram_tensor("xt", [D, SEQ], BF16, kind="ExternalInput")
    xn = nc.dram_tensor("xn", [SEQ, D], BF16, kind="ExternalInput")
    wqT = nc.dram_tensor("wqT", [D, D], BF16, kind="ExternalInput")
    wkN = nc.dram_tensor("wkN", [D, D], BF16, kind="ExternalInput")
    wvT = nc.dram_tensor("wvT", [D, D], BF16, kind="ExternalInput")
    mask_in = nc.dram_tensor("mask", [128, 1024], F32, kind="ExternalInput")
    out = nc.dram_tensor("out", [D, QLOC], F32, kind="ExternalOutput")
    io = (xq, xt, xn, wqT, wkN, wvT, out)

    with tile.TileContext(nc) as tc:
        with (
            tc.tile_pool(name="const", bufs=1) as cp,
            tc.tile_pool(name="psum", bufs=2, space="PSUM") as pp,
        ):
            ident = cp.tile([128, 128], BF16, name="ident")
            masks.make_identity(nc, ident[:])
            mask_sb = cp.tile([128, 1024], F32, name="mask_sb")
            nc.sync.dma_start(mask_sb[:], mask_in[:])
            for rep in range(reps):
                if rep > 0:
                    # serialize reps so the R-slope measures single-shot latency
                    tc.strict_bb_all_engine_barrier()
                _emit_compute(nc, tc, pp, (ident, mask_sb), io, rep, variant)

    nc.compile()
    return nc


def make_in_maps(x, Wq, Wk, Wv):
    x = np.asarray(x, dtype=np.float32)
    Wq = np.asarray(Wq, dtype=np.float32)
    Wk = np.asarray(Wk, dtype=np.float32)
    Wv = np.asarray(Wv, dtype=np.float32)

    bf = ml_dtypes.bfloat16
    xT = np.ascontiguousarray(x.T).astype(bf)          # [D, SEQ]
    xn = np.ascontiguousarray(x).astype(bf)            # [SEQ, D]
    wqT = np.ascontiguousarray(Wq.T).astype(bf)
    wkN = np.ascontiguousarray(Wk).astype(bf)          # row-major [dout, din]
    wvT = np.ascontiguousarray(Wv.T).astype(bf)

    p = np.arange(128)[:, None]
    j = np.arange(1024)[None, :]
    in_maps = []
    for i in CORE_IDS:
        mask_i = np.where(j <= 8 * p + i, 0.0, MASK_VAL).astype(np.float32)
        in_maps.append({
            "xq": np.ascontiguousarray(xT[:, i::N_CORES]),
            "xt": xT, "xn": xn,
            "wqT": wqT, "wkN": wkN, "wvT": wvT,
            "mask": mask_i,
        })
    return in_maps


def assemble(results):
    out = np.empty((SEQ, D), dtype=np.float32)
    for i in CORE_IDS:
        out[i::N_CORES] = results[i]["out"].T
    return out


def kernel(x, Wq, Wk, Wv):
    global _NC_CACHE
    if _NC_CACHE is None:
        _NC_CACHE = build_nc()
    in_maps = make_in_maps(x, Wq, Wk, Wv)
    res = run_bass_kernel_spmd(nc := _NC_CACHE, in_maps, core_ids=CORE_IDS)
    return assemble(res.results)


_NC_CACHE = None

